# revision 19
# baseline (speedup 1.0000x reference)
"""Masked-L1 depth loss on 8 TRN2 NeuronCores.

loss = sum(|output - label0| * label1) / count_nonzero(label0)

Data-parallel: batch dim (16) sharded 2-per-core. Each core streams its
[128, 15360] f32 shard view and produces per-partition partial sums
(loss and nonzero count) in a [128, 2*T] tile summed on host.

Full-SBUF residence: all three shard tensors (3 x 60 KiB/partition
= 180 KiB < ~208 KiB usable) live in SBUF simultaneously, so there are
ZERO WAR deps on input buffers: all 24 input DMAs are dispatched
unconditionally up-front on the SP HWDGE ring in strict consumption
order (b, a, c per tile). The SDMA engines then drain the whole 23.6 MB
stream back-to-back at the HBM read rate (~420 GB/s measured, i.e.
~26.5 GB/s on each of the 16 SDMA engines) regardless of compute
jitter; dispatch is never gated on compute buffer-frees.

Known residual: on 1-4 (usually even-numbered) cores per run, one SDMA
engine -- most often engine 0 or 15 -- runs ~20% slow on its HBM read
path (per-descriptor latency jitter, environmental, confirmed
schedule-independent across buffering/queue/dtype variants). That
engine's 1/16 byte share then sets an ~11 us longer stream tail on the
afflicted core, which bounds the max-core exec time at ~83-92 us while
healthy cores run ~73-75 us (stream ~57 us + ~2 us lead-in + ~4 us
drain/output + ~7 us fixed compiler sem-reset postamble).

Per [128, F] tile (elementwise ops in-place on the resident tensors):
  DVE: a <- a - b (TT), c <- a * c (TT), b <- (b != 0) (TS)
  ACT: |c| with fused row-sum accum (loss partials; valid since label1>=0
       so |d|*c == |d*c|), then Copy(b) with fused row-sum accum (count).
Tile sizes ramp up then down: small first tile lets compute start early,
progressively smaller tail tiles collapse the end-of-stream drain.
"""

import time

import numpy as np

import concourse.bacc as bacc
import concourse.mybir as mybir
from concourse import tile
from concourse.bass_utils import run_bass_kernel_spmd
from concourse.tile_rust import add_dep_helper

N_CORES = 8
P = 128
B, C, H, W = 16, 15, 256, 256
TOTAL = B * C * H * W                  # 15728640
PER_CORE = TOTAL // N_CORES            # 1966080
FREE = PER_CORE // P                   # 15360
TILE_SIZES = [640, 2560, 2560, 2560, 2560, 2560, 1280, 640]
assert sum(TILE_SIZES) == FREE
N_TILES = len(TILE_SIZES)

_nc_cache = None


def build_nc():
    global _nc_cache
    if _nc_cache is not None:
        return _nc_cache
    nc = bacc.Bacc("TRN2", target_bir_lowering=False, debug=False)
    f32 = mybir.dt.float32
    a = nc.dram_tensor("output", [P, FREE], f32, kind="ExternalInput").ap()
    b = nc.dram_tensor("label0", [P, FREE], f32, kind="ExternalInput").ap()
    c = nc.dram_tensor("label1", [P, FREE], f32, kind="ExternalInput").ap()
    o = nc.dram_tensor("out", [P, 2 * N_TILES], f32, kind="ExternalOutput").ap()

    sub = mybir.AluOpType.subtract
    mult = mybir.AluOpType.mult
    neq = mybir.AluOpType.not_equal

    with tile.TileContext(nc) as tc:
        with (
            tc.tile_pool(name="data", bufs=1) as data_pool,
            tc.tile_pool(name="acc", bufs=1) as acc_pool,
        ):
            at = data_pool.tile([P, FREE], f32, tag="a")
            bt = data_pool.tile([P, FREE], f32, tag="b")
            ct = data_pool.tile([P, FREE], f32, tag="c")
            acc = acc_pool.tile([P, 2 * N_TILES], f32)

            # all input DMAs up-front, in consumption order; nothing ever
            # blocks these (no buffer reuse), so the SP ring stays full
            slices = []
            off = 0
            for ft in TILE_SIZES:
                sl = slice(off, off + ft)
                slices.append(sl)
                off += ft
                nc.sync.dma_start(bt[:, sl], b[:, sl])
                nc.sync.dma_start(at[:, sl], a[:, sl])
                nc.sync.dma_start(ct[:, sl], c[:, sl])

            for t, sl in enumerate(slices):
                nc.vector.tensor_tensor(at[:, sl], at[:, sl], bt[:, sl], sub)
                mul_i = nc.vector.tensor_tensor(ct[:, sl], at[:, sl], ct[:, sl], mult)
                last = t == N_TILES - 1
                if last:
                    # final tile: fused DVE neq+row-sum (runs parallel to
                    # ACT's abs) so the tail has one ACT pass, not two
                    neq_i = nc.vector.tensor_scalar(
                        bt[:, sl], bt[:, sl], 0.0, None, neq,
                        mybir.AluOpType.add,
                        accum_out=acc[:, N_TILES + t : N_TILES + t + 1])
                else:
                    neq_i = nc.vector.tensor_scalar(
                        bt[:, sl], bt[:, sl], 0.0, None, neq)
                # keep DVE order sub -> mul -> neq so ACT's abs (gated on
                # mul) starts as early as possible each iteration
                add_dep_helper(neq_i.ins, mul_i.ins, sync=False,
                               reason="order neq after mul on DVE")
                nc.scalar.activation(
                    at[:, sl],
                    ct[:, sl],
                    mybir.ActivationFunctionType.Abs,
                    accum_out=acc[:, t : t + 1],
                )
                if not last:
                    nc.scalar.activation(
                        ct[:, sl],
                        bt[:, sl],
                        mybir.ActivationFunctionType.Copy,
                        accum_out=acc[:, N_TILES + t : N_TILES + t + 1],
                    )
            nc.scalar.dma_start(o[:, :], acc[:, :])
    nc.compile()
    _nc_cache = nc
    return nc


def run_cores(output, label0, label1, **spmd_kwargs):
    """Shard, run the 8-core SPMD kernel, return BassKernelResults."""
    nc = build_nc()
    shards = {}
    for name, arr in (("output", output), ("label0", label0), ("label1", label1)):
        arr = np.ascontiguousarray(np.asarray(arr, dtype=np.float32))
        shards[name] = arr.reshape(N_CORES, P, FREE)
    in_maps = [
        {name: shards[name][i] for name in shards} for i in range(N_CORES)
    ]
    last_err = None
    for attempt in range(3):
        try:
            return run_bass_kernel_spmd(
                nc, in_maps, core_ids=list(range(N_CORES)), **spmd_kwargs
            )
        except Exception as e:  # transient NRT device-unrecoverable blips
            last_err = e
            if "UNRECOVERABLE" not in str(e) and "UNAVAILABLE" not in str(e):
                raise
            time.sleep(5)
    raise last_err


def kernel(output, label0, label1):
    res = run_cores(output, label0, label1)
    loss = 0.0
    cnt = 0.0
    for r in res.results:
        part = np.asarray(r["out"], dtype=np.float64)
        loss += part[:, :N_TILES].sum()
        cnt += part[:, N_TILES:].sum()
    cnt = int(round(cnt))
    if cnt == 0:
        val = np.float32(0.0)
    else:
        val = np.float32(np.float32(loss) / np.float32(cnt))
    return np.asarray(val, dtype=np.float32)


# revision 20
# speedup vs baseline: 1.1098x; 1.1098x over previous
"""Masked-L1 depth loss on 8 TRN2 NeuronCores.

loss = sum(|output - label0| * label1) / count_nonzero(label0)

Data-parallel: batch dim (16) sharded 2-per-core. Each core streams its
[128, 15360] f32 shard view and produces per-partition partial sums
(loss and nonzero count) in a [128, 2*T] tile summed on host.

Full-SBUF residence: all three shard tensors (3 x 60 KiB/partition
= 180 KiB < ~208 KiB usable) live in SBUF simultaneously, so there are
ZERO WAR deps on input buffers: all 24 input DMAs are dispatched
unconditionally up-front on the SP HWDGE ring in strict consumption
order (b, a, c per tile). The SDMA engines then drain the whole 23.6 MB
stream back-to-back at the HBM read rate (~420 GB/s measured, i.e.
~26.5 GB/s on each of the 16 SDMA engines) regardless of compute
jitter; dispatch is never gated on compute buffer-frees.

Known residual: on 1-4 (usually even-numbered) cores per run, one SDMA
engine -- most often engine 0 or 15 -- runs ~20% slow on its HBM read
path (per-descriptor latency jitter, environmental, confirmed
schedule-independent across buffering/queue/dtype variants). That
engine's 1/16 byte share then sets an ~11 us longer stream tail on the
afflicted core, which bounds the max-core exec time at ~83-92 us while
healthy cores run ~73-75 us (stream ~57 us + ~2 us lead-in + ~4 us
drain/output + ~7 us fixed compiler sem-reset postamble).

Per [128, F] tile (elementwise ops in-place on the resident tensors):
  DVE: a <- a - b (TT), c <- a * c (TT), b <- (b != 0) (TS)
  ACT: |c| with fused row-sum accum (loss partials; valid since label1>=0
       so |d|*c == |d*c|), then Copy(b) with fused row-sum accum (count).
Tile sizes ramp up then down: small first tile lets compute start early,
progressively smaller tail tiles collapse the end-of-stream drain.
"""

import time

import numpy as np

import concourse.bacc as bacc
import concourse.mybir as mybir
from concourse import tile
from concourse.bass_utils import run_bass_kernel_spmd
from concourse.tile_rust import add_dep_helper

N_CORES = 8
P = 128
B, C, H, W = 16, 15, 256, 256
TOTAL = B * C * H * W                  # 15728640
PER_CORE = TOTAL // N_CORES            # 1966080
FREE = PER_CORE // P                   # 15360
TILE_SIZES = [1280, 2560, 2560, 2560, 2560, 1920, 1280, 640]
assert sum(TILE_SIZES) == FREE
N_TILES = len(TILE_SIZES)

_nc_cache = None


def build_nc():
    global _nc_cache
    if _nc_cache is not None:
        return _nc_cache
    nc = bacc.Bacc("TRN2", target_bir_lowering=False, debug=False)
    f32 = mybir.dt.float32
    a = nc.dram_tensor("output", [P, FREE], f32, kind="ExternalInput").ap()
    b = nc.dram_tensor("label0", [P, FREE], f32, kind="ExternalInput").ap()
    c = nc.dram_tensor("label1", [P, FREE], f32, kind="ExternalInput").ap()
    o = nc.dram_tensor("out", [P, 2 * N_TILES], f32, kind="ExternalOutput").ap()

    sub = mybir.AluOpType.subtract
    mult = mybir.AluOpType.mult
    neq = mybir.AluOpType.not_equal

    with tile.TileContext(nc) as tc:
        with (
            tc.tile_pool(name="data", bufs=1) as data_pool,
            tc.tile_pool(name="acc", bufs=1) as acc_pool,
        ):
            at = data_pool.tile([P, FREE], f32, tag="a")
            bt = data_pool.tile([P, FREE], f32, tag="b")
            ct = data_pool.tile([P, FREE], f32, tag="c")
            acc = acc_pool.tile([P, 2 * N_TILES], f32)

            # all input DMAs up-front, in consumption order; nothing ever
            # blocks these (no buffer reuse), so the SP ring stays full
            slices = []
            off = 0
            for ft in TILE_SIZES:
                sl = slice(off, off + ft)
                slices.append(sl)
                off += ft
                nc.sync.dma_start(bt[:, sl], b[:, sl])
                nc.sync.dma_start(at[:, sl], a[:, sl])
                nc.sync.dma_start(ct[:, sl], c[:, sl])

            for t, sl in enumerate(slices):
                nc.vector.tensor_tensor(at[:, sl], at[:, sl], bt[:, sl], sub)
                mul_i = nc.vector.tensor_tensor(ct[:, sl], at[:, sl], ct[:, sl], mult)
                last = t == N_TILES - 1
                if last:
                    # final tile: fused DVE neq+row-sum (runs parallel to
                    # ACT's abs) so the tail has one ACT pass, not two
                    neq_i = nc.vector.tensor_scalar(
                        bt[:, sl], bt[:, sl], 0.0, None, neq,
                        mybir.AluOpType.add,
                        accum_out=acc[:, N_TILES + t : N_TILES + t + 1])
                else:
                    neq_i = nc.vector.tensor_scalar(
                        bt[:, sl], bt[:, sl], 0.0, None, neq)
                # keep DVE order sub -> mul -> neq so ACT's abs (gated on
                # mul) starts as early as possible each iteration
                add_dep_helper(neq_i.ins, mul_i.ins, sync=False,
                               reason="order neq after mul on DVE")
                nc.scalar.activation(
                    at[:, sl],
                    ct[:, sl],
                    mybir.ActivationFunctionType.Abs,
                    accum_out=acc[:, t : t + 1],
                )
                if not last:
                    nc.scalar.activation(
                        ct[:, sl],
                        bt[:, sl],
                        mybir.ActivationFunctionType.Copy,
                        accum_out=acc[:, N_TILES + t : N_TILES + t + 1],
                    )
            nc.scalar.dma_start(o[:, :], acc[:, :])
    nc.compile()
    _nc_cache = nc
    return nc


def run_cores(output, label0, label1, **spmd_kwargs):
    """Shard, run the 8-core SPMD kernel, return BassKernelResults."""
    nc = build_nc()
    shards = {}
    for name, arr in (("output", output), ("label0", label0), ("label1", label1)):
        arr = np.ascontiguousarray(np.asarray(arr, dtype=np.float32))
        shards[name] = arr.reshape(N_CORES, P, FREE)
    in_maps = [
        {name: shards[name][i] for name in shards} for i in range(N_CORES)
    ]
    last_err = None
    for attempt in range(3):
        try:
            return run_bass_kernel_spmd(
                nc, in_maps, core_ids=list(range(N_CORES)), **spmd_kwargs
            )
        except Exception as e:  # transient NRT device-unrecoverable blips
            last_err = e
            if "UNRECOVERABLE" not in str(e) and "UNAVAILABLE" not in str(e):
                raise
            time.sleep(5)
    raise last_err


def kernel(output, label0, label1):
    res = run_cores(output, label0, label1)
    loss = 0.0
    cnt = 0.0
    for r in res.results:
        part = np.asarray(r["out"], dtype=np.float64)
        loss += part[:, :N_TILES].sum()
        cnt += part[:, N_TILES:].sum()
    cnt = int(round(cnt))
    if cnt == 0:
        val = np.float32(0.0)
    else:
        val = np.float32(np.float32(loss) / np.float32(cnt))
    return np.asarray(val, dtype=np.float32)
